# revision 28
# baseline (speedup 1.0000x reference)
"""Trainium2 Bass kernel for nn_EnergyFromMaps (gnn_message_passing).

Strategy (8 NeuronCores, data-parallel):
  - B=4 sets x 1024 points; each core takes 512 points of one set and the
    full 9216-point context of that set (core c -> set c//2, half c%2).
  - Pair terms: the rectangle-overlap radius r = dist * (A^10+B^10)^(-1/10)
    with A=|u|/hw, B=|v|/hl where u = dist*cos(theta-a), v = dist*sin(theta-a)
    are *linear* in the context coords ->  computed as K=3 matmuls on the PE.
    dist cancels in dist/(r_i+r_j), so no sqrt/arctan2/div-by-dist at all.
  - d2 is computed in direct form (p-c)^2 (the matmul form cancels
    catastrophically near d2=0 and breaks the self-pair gate).
  - Map energies: bilinear/trilinear lookups via one indirect-DMA gather of
    28 values per point (4 corners x (pos + 2 classes x 3 maps)).
  - Host only shards/reshapes and adds the 8 partial sums.
"""

import numpy as np

import concourse.bass as bass
import concourse.bacc as bacc
import concourse.tile as tile
import concourse.mybir as mybir
from concourse.masks import make_identity

f32 = mybir.dt.float32
i32 = mybir.dt.int32
Alu = mybir.AluOpType
Act = mybir.ActivationFunctionType
Axis = mybir.AxisListType

PI = float(np.pi)
P = 128
NPTS = 512           # points per core
NT = NPTS // P       # 4 i-tiles
NCTX = 9216          # context points per set
TT = NCTX // P       # 72 ctx transpose tiles
F = 512              # j-tile width
NJ = NCTX // F       # 18 j-tiles
PLANE = 256 * 256
NPLANES = 98         # pem(1) + width(32) + length(32) + angle(32) + zero pad(1)
NMAP = NPLANES * PLANE
NCORES = 8
TINY = 1e-38         # keeps t = A^10+B^10 away from 0 (self pairs) -> no Inf


def _ap(handle_ap, offset, pattern):
    return bass.AP(tensor=handle_ap.tensor, offset=offset, ap=pattern)


def build_kernel():
    nc = bacc.Bacc("TRN2", target_bir_lowering=False, debug=False,
                   enable_asserts=False, num_devices=NCORES)

    pts_d = nc.dram_tensor("pts", [NPTS, 5], f32, kind="ExternalInput").ap()
    pm_d = nc.dram_tensor("pm", [NPTS], f32, kind="ExternalInput").ap()
    ctx_d = nc.dram_tensor("ctx", [NCTX, 5], f32, kind="ExternalInput").ap()
    cm_d = nc.dram_tensor("cm", [NCTX], f32, kind="ExternalInput").ap()
    gmap_d = nc.dram_tensor("gmap", [NMAP], f32, kind="ExternalInput").ap()
    cw_d = nc.dram_tensor("cw", [8], f32, kind="ExternalInput").ap()
    en_d = nc.dram_tensor("energies", [NPTS], f32, kind="ExternalOutput").ap()
    es_d = nc.dram_tensor("esum", [1, 1], f32, kind="ExternalOutput").ap()
    rows_dram = nc.dram_tensor("rows_scratch", [5, NCTX], f32, kind="Internal").ap()

    with tile.TileContext(nc) as tc:
        _emit(nc, tc, pts_d, pm_d, ctx_d, cm_d, gmap_d, cw_d, en_d, es_d,
              rows_dram)
    nc.compile()
    return nc


# minimax polys on [-pi/2, pi/2]:  sin(a)=cos(r), cos(a)=-sin(r), r=a-pi/2
_SC = (9.999994700e-01, -1.666589120e-01, 8.315962953e-03, -1.860891081e-04)
_CC = (9.999999673e-01, -4.999992687e-01, 4.166409061e-02, -1.385741578e-03,
       2.323749701e-05)


def _sincos(nc, pool, x, n, sin_dst, cos_dst, tag):
    """sin/cos for x in [0, pi) via DVE polynomials (ACT Sin crashes HW)."""
    r = pool.tile([P, n], f32, tag=tag + "_r")
    nc.vector.tensor_scalar(r[:], x, -PI / 2, None, Alu.add)
    u = pool.tile([P, n], f32, tag=tag + "_u")
    nc.vector.tensor_mul(u[:], r[:], r[:])
    p = pool.tile([P, n], f32, tag=tag + "_p")
    nc.vector.tensor_scalar(p[:], u[:], _CC[4], _CC[3], Alu.mult, Alu.add)
    nc.vector.tensor_mul(p[:], p[:], u[:])
    nc.vector.tensor_scalar(p[:], p[:], _CC[2], None, Alu.add)
    nc.vector.tensor_mul(p[:], p[:], u[:])
    nc.vector.tensor_scalar(p[:], p[:], _CC[1], None, Alu.add)
    nc.vector.tensor_mul(p[:], p[:], u[:])
    nc.vector.tensor_scalar(sin_dst, p[:], _CC[0], None, Alu.add)
    q = pool.tile([P, n], f32, tag=tag + "_q")
    nc.vector.tensor_scalar(q[:], u[:], -_SC[3], -_SC[2], Alu.mult, Alu.add)
    nc.vector.tensor_mul(q[:], q[:], u[:])
    nc.vector.tensor_scalar(q[:], q[:], -_SC[1], None, Alu.add)
    nc.vector.tensor_mul(q[:], q[:], u[:])
    nc.vector.tensor_scalar(q[:], q[:], -_SC[0], None, Alu.add)
    nc.vector.tensor_mul(cos_dst, q[:], r[:])


def _floor(nc, pool, q, n):
    """floor for q >= 0 (any convert rounding mode); returns a fresh tile."""
    fi = pool.tile([P, n], i32, tag="flr_i")
    ff = pool.tile([P, n], f32, tag="flr_f")
    dg = pool.tile([P, n], f32, tag="flr_d")
    nc.vector.tensor_copy(fi[:], q)
    nc.vector.tensor_copy(ff[:], fi[:])
    nc.vector.tensor_tensor(dg[:], ff[:], q, Alu.is_gt)
    out = pool.tile([P, n], f32, tag="flr_o")
    nc.vector.tensor_sub(out[:], ff[:], dg[:])
    return out


def _emit(nc, tc, pts_d, pm_d, ctx_d, cm_d, gmap_d, cw_d, en_d, es_d,
          rows_dram):
    from contextlib import ExitStack
    st = ExitStack()
    sg = st.enter_context(tc.tile_pool(name="singles", bufs=1))
    sc = st.enter_context(tc.tile_pool(name="scratch", bufs=2))
    bc = st.enter_context(tc.tile_pool(name="bcast", bufs=2))
    pp = st.enter_context(tc.tile_pool(name="pair", bufs=1))
    pmm = st.enter_context(tc.tile_pool(name="psum_mm", bufs=1, space="PSUM"))
    ptr = st.enter_context(tc.tile_pool(name="psum_tr", bufs=2, space="PSUM"))
    try:
        _emit_body(nc, tc, sg, sc, bc, pp, pmm, ptr, pts_d, pm_d, ctx_d,
                   cm_d, gmap_d, cw_d, en_d, es_d, rows_dram)
    finally:
        st.close()


def _emit_body(nc, tc, sg, sc, bc, pp, pmm, ptr, pts_d, pm_d, ctx_d, cm_d,
               gmap_d, cw_d, en_d, es_d, rows_dram):

    # ---------------- load + clip points ----------------
    pts_sb = sg.tile([P, NT, 5], f32)
    nc.sync.dma_start(out=pts_sb[:], in_=pts_d.rearrange("(t p) d -> p t d", p=P))
    pm_sb = sg.tile([P, NT], f32)
    nc.sync.dma_start(out=pm_sb[:], in_=pm_d.rearrange("(p t) -> p t", t=NT))

    y = pts_sb[:, :, 0]
    x = pts_sb[:, :, 1]
    w = pts_sb[:, :, 2]
    l = pts_sb[:, :, 3]
    a = pts_sb[:, :, 4]
    # check_inbound_state: clip y,x,w,l; wrap angle mod pi
    nc.vector.tensor_scalar(y, y, 0.0, 256.0, Alu.max, Alu.min)
    nc.vector.tensor_scalar(x, x, 0.0, 256.0, Alu.max, Alu.min)
    nc.vector.tensor_scalar(w, w, 2.0, 32.0, Alu.max, Alu.min)
    nc.vector.tensor_scalar(l, l, 2.0, 32.0, Alu.max, Alu.min)
    qa = sc.tile([P, NT], f32, tag="qa")
    nc.vector.tensor_scalar(qa[:], a, 1.0 / PI, None, Alu.mult)
    qf = _floor(nc, sc, qa[:], NT)
    nc.vector.scalar_tensor_tensor(a, qf[:], -PI, a, Alu.mult, Alu.add)

    # ---------------- per-point derived (qm -> lhsT bank) ----------------
    # qm cols (groups of 3 -> one PE transpose each, partition-aligned):
    #   u:[0 alpha, 1 cos_a, 2 sin_a]  v:[3 beta, 4 -sin_a, 5 cos_a]
    #   up:[6 p0, 7 p1, 8 ones]        vp:[9 p1, 10 -p0, 11 ones]
    qm = sg.tile([P, NT, 12], f32)
    _sincos(nc, sc, a, NT, qm[:, :, 2], qm[:, :, 1], "scp")
    t1 = sc.tile([P, NT], f32, tag="t1")
    t2 = sc.tile([P, NT], f32, tag="t2")
    nc.vector.tensor_mul(t1[:], y, qm[:, :, 1])
    nc.vector.tensor_mul(t2[:], x, qm[:, :, 2])
    nc.vector.tensor_add(qm[:, :, 0], t1[:], t2[:])          # alpha
    t3 = sc.tile([P, NT], f32, tag="t1")
    t4 = sc.tile([P, NT], f32, tag="t2")
    nc.vector.tensor_mul(t3[:], x, qm[:, :, 1])
    nc.vector.tensor_mul(t4[:], y, qm[:, :, 2])
    nc.vector.tensor_sub(qm[:, :, 3], t3[:], t4[:])          # beta
    nc.vector.tensor_scalar(qm[:, :, 4], qm[:, :, 2], -1.0, None, Alu.mult)
    nc.vector.tensor_copy(qm[:, :, 5], qm[:, :, 1])
    nc.vector.tensor_copy(qm[:, :, 6], y)
    nc.vector.tensor_copy(qm[:, :, 7], x)
    nc.vector.memset(qm[:, :, 8], 1.0)
    nc.vector.tensor_copy(qm[:, :, 9], x)
    nc.vector.tensor_scalar(qm[:, :, 10], y, -1.0, None, Alu.mult)
    nc.vector.memset(qm[:, :, 11], 1.0)

    rhw = sg.tile([P, NT], f32)
    rhl = sg.tile([P, NT], f32)
    th = sc.tile([P, NT], f32, tag="t1")
    nc.vector.tensor_scalar(th[:], w, 4.0, None, Alu.mult)
    nc.vector.reciprocal(rhw[:], th[:])
    th2 = sc.tile([P, NT], f32, tag="t2")
    nc.vector.tensor_scalar(th2[:], l, 4.0, None, Alu.mult)
    nc.vector.reciprocal(rhl[:], th2[:])
    neg_a = sg.tile([P, NT], f32)
    nc.vector.tensor_scalar(neg_a[:], a, -1.0, None, Alu.mult)

    # area energy (pre-mask): 1 - 2*min(w*l/256, 1)
    ae = sg.tile([P, NT], f32)
    t5 = sc.tile([P, NT], f32, tag="t1")
    nc.vector.tensor_mul(t5[:], w, l)
    nc.vector.tensor_scalar(t5[:], t5[:], 1.0 / 256.0, 1.0, Alu.mult, Alu.min)
    nc.vector.tensor_scalar(ae[:], t5[:], -2.0, 1.0, Alu.mult, Alu.add)

    # ---------------- lhsT bank via PE transpose ----------------
    identity = sg.tile([P, P], f32)
    make_identity(nc, identity[:])
    # PE operand windows: lhsT and rhs must share the same base partition
    # (0/32/64 quadrants).  Pack into shared tensors to save SBUF free range:
    #   lhs1: u:[alpha,cos,sin]@0  up:[p0,p1,1]@32  vp:[p1,-p0,1]@64
    #   lhs2: v:[beta,-sin,cos]@0
    lhs1 = sg.tile([67, NPTS], f32)
    lhs2 = sg.tile([3, NPTS], f32)
    lhs_dst = [lhs1[0:3, :], lhs2[0:3, :], lhs1[32:35, :], lhs1[64:67, :]]
    for t in range(NT):
        cols = slice(t * P, (t + 1) * P)
        for g in range(4):
            ps = ptr.tile([3, P], f32, tag="tr3")
            nc.tensor.transpose(ps[:], qm[:, t, 3 * g:3 * g + 3], identity[:])
            nc.vector.tensor_copy(lhs_dst[g][:, cols], ps[:])

    # ---------------- load + clip context ----------------
    ctx_sb = sg.tile([P, TT, 5], f32)
    nc.sync.dma_start(out=ctx_sb[:], in_=ctx_d.rearrange("(t p) d -> p t d", p=P))
    cy = ctx_sb[:, :, 0]
    cx = ctx_sb[:, :, 1]
    cwd = ctx_sb[:, :, 2]
    cl = ctx_sb[:, :, 3]
    ca = ctx_sb[:, :, 4]
    nc.vector.tensor_scalar(cy, cy, 0.0, 256.0, Alu.max, Alu.min)
    nc.vector.tensor_scalar(cx, cx, 0.0, 256.0, Alu.max, Alu.min)
    nc.vector.tensor_scalar(cwd, cwd, 2.0, 32.0, Alu.max, Alu.min)
    nc.vector.tensor_scalar(cl, cl, 2.0, 32.0, Alu.max, Alu.min)
    qca = sc.tile([P, TT], f32, tag="qca")
    nc.vector.tensor_scalar(qca[:], ca, 1.0 / PI, None, Alu.mult)
    qcf = _floor(nc, sc, qca[:], TT)
    nc.vector.scalar_tensor_tensor(ca, qcf[:], -PI, ca, Alu.mult, Alu.add)

    # qc cols (aligned transpose groups):
    #   uv:[0 ones, 1 -c0, 2 -c1]  up:[3 cosj, 4 sinj, 5 -gam]
    #   vp:[6 cosj, 7 sinj, 8 -del]  stag:[9 rhwj, 10 rhlj, 11 c0, 12 c1, 13 aj]
    qc = sg.tile([P, TT, 14], f32)
    nc.vector.memset(qc[:, :, 0], 1.0)
    nc.vector.tensor_scalar(qc[:, :, 1], cy, -1.0, None, Alu.mult)
    nc.vector.tensor_scalar(qc[:, :, 2], cx, -1.0, None, Alu.mult)
    _sincos(nc, sc, ca, TT, qc[:, :, 4], qc[:, :, 3], "scc")
    nc.vector.tensor_copy(qc[:, :, 6], qc[:, :, 3])
    nc.vector.tensor_copy(qc[:, :, 7], qc[:, :, 4])
    u1 = sc.tile([P, TT], f32, tag="u1")
    u2 = sc.tile([P, TT], f32, tag="u2")
    # -gam = (-c0)*cos + (-c1)*sin
    nc.vector.tensor_mul(u1[:], qc[:, :, 1], qc[:, :, 3])
    nc.vector.tensor_mul(u2[:], qc[:, :, 2], qc[:, :, 4])
    nc.vector.tensor_add(qc[:, :, 5], u1[:], u2[:])
    # -del = (-c1)*cos + c0*sin
    u3 = sc.tile([P, TT], f32, tag="u1")
    u4 = sc.tile([P, TT], f32, tag="u2")
    nc.vector.tensor_mul(u3[:], qc[:, :, 2], qc[:, :, 3])
    nc.vector.tensor_mul(u4[:], cy, qc[:, :, 4])
    nc.vector.tensor_add(qc[:, :, 8], u3[:], u4[:])
    u5 = sc.tile([P, TT], f32, tag="u1")
    nc.vector.tensor_scalar(u5[:], cwd, 4.0, None, Alu.mult)
    nc.vector.reciprocal(qc[:, :, 9], u5[:])
    u6 = sc.tile([P, TT], f32, tag="u2")
    nc.vector.tensor_scalar(u6[:], cl, 4.0, None, Alu.mult)
    nc.vector.reciprocal(qc[:, :, 10], u6[:])
    nc.vector.tensor_copy(qc[:, :, 11], cy)
    nc.vector.tensor_copy(qc[:, :, 12], cx)
    nc.vector.tensor_copy(qc[:, :, 13], ca)

    # rhs bank: [ones,-c0,-c1]@0  [cosj,sinj,-gam]@32  [cosj,sinj,-del]@64
    rows_all = sg.tile([67, NCTX], f32)
    stag = sg.tile([5, NCTX], f32)
    rows_dst = [rows_all[0:3, :], rows_all[32:35, :], rows_all[64:67, :]]
    for t in range(TT):
        cols = slice(t * P, (t + 1) * P)
        for g in range(3):
            ps = ptr.tile([3, P], f32, tag="tr3")
            nc.tensor.transpose(ps[:], qc[:, t, 3 * g:3 * g + 3], identity[:])
            nc.vector.tensor_copy(rows_dst[g][:, cols], ps[:])
        ps5 = ptr.tile([5, P], f32, tag="tr5")
        nc.tensor.transpose(ps5[:], qc[:, t, 9:14], identity[:])
        nc.vector.tensor_copy(stag[0:5, cols], ps5[:])
    # stag rows: 0 rhwj, 1 rhlj, 2 c0, 3 c1, 4 aj  -> DRAM for broadcasts
    nc.sync.dma_start(out=rows_dram, in_=stag[:])

    # ---------------- map gather indices + weights ----------------
    ypix = sc.tile([P, NT], f32, tag="ypix")
    xpix = sc.tile([P, NT], f32, tag="xpix")
    nc.vector.tensor_scalar(ypix[:], y, 255.0, None, Alu.min)
    nc.vector.tensor_scalar(xpix[:], x, 255.0, None, Alu.min)
    iy0 = _floor(nc, sc, ypix[:], NT)
    wy = sg.tile([P, NT], f32)
    nc.vector.tensor_sub(wy[:], ypix[:], iy0[:])
    ix0 = _floor(nc, sc, xpix[:], NT)
    wx = sg.tile([P, NT], f32)
    nc.vector.tensor_sub(wx[:], xpix[:], ix0[:])
    bpos = sc.tile([P, NT], f32, tag="bpos")
    nc.vector.scalar_tensor_tensor(bpos[:], iy0[:], 256.0, ix0[:],
                                   Alu.mult, Alu.add)

    # corner weights W: [(1-wy)(1-wx), (1-wy)wx, wy(1-wx), wy*wx]
    Wt = sg.tile([P, NT, 4], f32)
    omy = sc.tile([P, NT], f32, tag="t1")
    omx = sc.tile([P, NT], f32, tag="t2")
    nc.vector.tensor_scalar(omy[:], wy[:], -1.0, 1.0, Alu.mult, Alu.add)
    nc.vector.tensor_scalar(omx[:], wx[:], -1.0, 1.0, Alu.mult, Alu.add)
    nc.vector.tensor_mul(Wt[:, :, 0], omy[:], omx[:])
    nc.vector.tensor_mul(Wt[:, :, 1], omy[:], wx[:])
    nc.vector.tensor_mul(Wt[:, :, 2], wy[:], omx[:])
    nc.vector.tensor_mul(Wt[:, :, 3], wy[:], wx[:])

    # class index helper: returns (c0f, c1f, cwt) for value tile v
    def klass(v, scale_, bias_, cyclic, cw_tag):
        cf = sc.tile([P, NT], f32, tag=cw_tag + "_cf")
        nc.vector.tensor_scalar(cf[:], v, scale_, bias_, Alu.mult, Alu.add)
        if cyclic:
            lt = sc.tile([P, NT], f32, tag=cw_tag + "_lt")
            nc.vector.tensor_scalar(lt[:], cf[:], 0.0, None, Alu.is_lt)
            nc.vector.scalar_tensor_tensor(cf[:], lt[:], 32.0, cf[:],
                                           Alu.mult, Alu.add)
        else:
            nc.vector.tensor_scalar(cf[:], cf[:], 0.0, 31.0, Alu.max, Alu.min)
        c0f = _floor(nc, sc, cf[:], NT)
        cwt = sg.tile([P, NT], f32, tag=cw_tag)
        nc.vector.tensor_sub(cwt[:], cf[:], c0f[:])
        c1f = sc.tile([P, NT], f32, tag=cw_tag + "_c1")
        if cyclic:
            nc.vector.tensor_scalar(c1f[:], c0f[:], 1.0, None, Alu.add)
            eq = sc.tile([P, NT], f32, tag=cw_tag + "_eq")
            nc.vector.tensor_scalar(eq[:], c1f[:], 32.0, None, Alu.is_equal)
            nc.vector.scalar_tensor_tensor(c1f[:], eq[:], -32.0, c1f[:],
                                           Alu.mult, Alu.add)
        else:
            nc.vector.tensor_scalar(c1f[:], c0f[:], 1.0, 31.0, Alu.add, Alu.min)
        # keep c0f alive: copy into persistent tag
        c0k = sc.tile([P, NT], f32, tag=cw_tag + "_c0")
        nc.vector.tensor_copy(c0k[:], c0f[:])
        return c0k, c1f, cwt

    c0w, c1w, cw_w = klass(w, 32.0 / 30.0, -2.0 * 32.0 / 30.0 - 0.5, False, "kw")
    c0l, c1l, cw_l = klass(l, 32.0 / 30.0, -2.0 * 32.0 / 30.0 - 0.5, False, "kl")
    c0a, c1a, cw_a = klass(a, 32.0 / PI, -0.5, True, "ka")

    # base indices (one per q in [pos, wA, wB, lA, lB, aA, aB]); the 4
    # bilinear corners come from 2-wide gather rows (+0,+1) at element
    # offsets 0 and 256.  HW indirect DMA consumes ONE index per
    # partition-row of the dest, so each (q, yrow) is its own gather.
    idxf = sg.tile([P, NT, 7], f32)
    nc.vector.tensor_copy(idxf[:, :, 0], bpos[:])
    # width planes start at 1, length at 33, angle at 65
    bb = sc.tile([P, NT], f32, tag="bb")
    for qi, plane0, cf in ((1, 1.0, c0w), (2, 1.0, c1w), (3, 33.0, c0l),
                          (4, 33.0, c1l), (5, 65.0, c0a), (6, 65.0, c1a)):
        nc.vector.tensor_scalar(bb[:], cf[:], float(plane0), None, Alu.add)
        nc.vector.scalar_tensor_tensor(idxf[:, :, qi], bb[:], 65536.0,
                                       bpos[:], Alu.mult, Alu.add)
    idx32 = sg.tile([P, NT, 7], i32)
    nc.vector.tensor_copy(idx32[:], idxf[:])

    # gathers: gath col layout q*4 + yy*2 + xx, addr = idx + 256*yy + xx
    gmap_view = _ap(gmap_d, 0, [[PLANE, NPLANES], [1, PLANE]])
    gath = sg.tile([P, NT, 28], f32)
    for t in range(NT):
        for q in range(7):
            for yy in range(2):
                col = q * 4 + 2 * yy
                nc.gpsimd.indirect_dma_start(
                    out=gath[:, t, col:col + 2],
                    out_offset=None,
                    in_=gmap_view,
                    in_offset=bass.IndirectOffsetOnAxis(
                        ap=idx32[:, t, q:q + 1], axis=1),
                    element_offset=256 * yy,
                )

    # ---------------- pair loop ----------------
    ovred = sg.tile([P, NT * NJ], f32)   # col = t*NJ + jt
    alred = sg.tile([P, NT * NJ], f32)

    for jt in range(NJ):
        j0 = jt * F
        c0r = bc.tile([P, F], f32, tag="c0r")
        c1r = bc.tile([P, F], f32, tag="c1r")
        ajr = bc.tile([P, F], f32, tag="ajr")
        cmr = bc.tile([P, F], f32, tag="cmr")
        rhwr = bc.tile([P, F], f32, tag="rhwr")
        rhlr = bc.tile([P, F], f32, tag="rhlr")
        nc.sync.dma_start(out=rhwr[:], in_=_ap(rows_dram, 0 * NCTX + j0, [[0, P], [1, F]]))
        nc.sync.dma_start(out=rhlr[:], in_=_ap(rows_dram, 1 * NCTX + j0, [[0, P], [1, F]]))
        nc.sync.dma_start(out=c0r[:], in_=_ap(rows_dram, 2 * NCTX + j0, [[0, P], [1, F]]))
        nc.sync.dma_start(out=c1r[:], in_=_ap(rows_dram, 3 * NCTX + j0, [[0, P], [1, F]]))
        nc.sync.dma_start(out=ajr[:], in_=_ap(rows_dram, 4 * NCTX + j0, [[0, P], [1, F]]))
        nc.sync.dma_start(out=cmr[:], in_=_ap(cm_d, j0, [[0, P], [1, F]]))

        for t in range(NT):
            icols = slice(t * P, (t + 1) * P)
            jcols = slice(j0, j0 + F)
            u_ps = pmm.tile([P, F], f32, tag="mm_u", space="PSUM")
            v_ps = pmm.tile([P, F], f32, tag="mm_v", space="PSUM")
            up_ps = pmm.tile([P, F], f32, tag="mm_up", space="PSUM")
            vp_ps = pmm.tile([P, F], f32, tag="mm_vp", space="PSUM")
            nc.tensor.matmul(out=u_ps[:], lhsT=lhs1[0:3, icols],
                             rhs=rows_all[0:3, jcols], start=True, stop=True)
            nc.tensor.matmul(out=v_ps[:], lhsT=lhs2[0:3, icols],
                             rhs=rows_all[0:3, jcols], start=True, stop=True)
            nc.tensor.matmul(out=up_ps[:], lhsT=lhs1[32:35, icols],
                             rhs=rows_all[32:35, jcols], start=True, stop=True)
            nc.tensor.matmul(out=vp_ps[:], lhsT=lhs1[64:67, icols],
                             rhs=rows_all[64:67, jcols], start=True, stop=True)

            p0c = pts_sb[:, t, 0:1]
            p1c = pts_sb[:, t, 1:2]
            # --- d2 direct form ---
            nd0 = pp.tile([P, F], f32, tag="nd0")
            nd1 = pp.tile([P, F], f32, tag="nd1")
            sq0 = pp.tile([P, F], f32, tag="sq0")
            nc.vector.tensor_scalar(nd0[:], c0r[:], p0c, None, Alu.subtract)
            nc.vector.tensor_scalar(nd1[:], c1r[:], p1c, None, Alu.subtract)
            nc.scalar.activation(sq0[:], nd0[:], Act.Square)
            nc.vector.tensor_mul(nd1[:], nd1[:], nd1[:])
            nc.gpsimd.tensor_add(sq0[:], sq0[:], nd1[:])     # sq0 = d2

            # --- point-side radius factor g_i ---
            A = pp.tile([P, F], f32, tag="A")
            B = pp.tile([P, F], f32, tag="B")
            nc.scalar.activation(A[:], u_ps[:], Act.Abs, scale=rhw[:, t:t + 1])
            nc.scalar.activation(B[:], v_ps[:], Act.Abs, scale=rhl[:, t:t + 1])
            A2 = pp.tile([P, F], f32, tag="A2")
            A4 = pp.tile([P, F], f32, tag="A4")
            nc.scalar.activation(A2[:], A[:], Act.Square)
            nc.scalar.activation(A4[:], A2[:], Act.Square)
            nc.scalar.activation(A4[:], A4[:], Act.Square)   # A8 in place
            nc.vector.tensor_mul(A[:], A4[:], A2[:])         # A10
            B2 = pp.tile([P, F], f32, tag="B2")
            B4 = pp.tile([P, F], f32, tag="B4")
            nc.scalar.activation(B2[:], B[:], Act.Square)
            nc.scalar.activation(B4[:], B2[:], Act.Square)
            nc.scalar.activation(B4[:], B4[:], Act.Square)   # B8
            nc.vector.tensor_mul(B[:], B4[:], B2[:])         # B10
            nc.vector.scalar_tensor_tensor(B[:], A[:], TINY, B[:],
                                           Alu.add, Alu.add)  # t_i
            nc.scalar.activation(B[:], B[:], Act.Ln)
            nc.scalar.activation(B[:], B[:], Act.Exp, scale=-0.1)  # g_i

            # --- context-side radius factor g_j ---
            Ap = pp.tile([P, F], f32, tag="Ap")
            Bp = pp.tile([P, F], f32, tag="Bp")
            nc.scalar.activation(Ap[:], up_ps[:], Act.Abs)
            nc.scalar.activation(Bp[:], vp_ps[:], Act.Abs)
            nc.vector.tensor_mul(Ap[:], Ap[:], rhwr[:])
            nc.vector.tensor_mul(Bp[:], Bp[:], rhlr[:])
            Ap2 = pp.tile([P, F], f32, tag="Ap2")
            Ap4 = pp.tile([P, F], f32, tag="Ap4")
            nc.vector.tensor_mul(Ap2[:], Ap[:], Ap[:])
            nc.vector.tensor_mul(Ap4[:], Ap2[:], Ap2[:])
            nc.vector.tensor_mul(Ap4[:], Ap4[:], Ap4[:])     # Ap8
            nc.vector.tensor_mul(Ap[:], Ap4[:], Ap2[:])      # Ap10
            Bp2 = pp.tile([P, F], f32, tag="Bp2")
            Bp4 = pp.tile([P, F], f32, tag="Bp4")
            nc.vector.tensor_mul(Bp2[:], Bp[:], Bp[:])
            nc.vector.tensor_mul(Bp4[:], Bp2[:], Bp2[:])
            nc.vector.tensor_mul(Bp4[:], Bp4[:], Bp4[:])     # Bp8
            nc.vector.tensor_mul(Bp[:], Bp4[:], Bp2[:])      # Bp10
            nc.vector.scalar_tensor_tensor(Bp[:], Ap[:], TINY, Bp[:],
                                           Alu.add, Alu.add)  # t_j
            nc.scalar.activation(Bp[:], Bp[:], Act.Ln)
            nc.scalar.activation(Bp[:], Bp[:], Act.Exp, scale=-0.1)  # g_j

            # --- overlap ---
            nc.gpsimd.tensor_add(B[:], B[:], Bp[:])          # G
            nc.vector.reciprocal(B[:], B[:])                 # r = 1/G
            nc.vector.tensor_scalar(B[:], B[:], -8.0, 1.0, Alu.mult, Alu.add)
            nonself = pp.tile([P, F], f32, tag="nonself")
            nc.vector.tensor_scalar(nonself[:], sq0[:], 0.0, None, Alu.is_gt)
            nc.vector.scalar_tensor_tensor(nonself[:], nonself[:],
                                           pm_sb[:, t:t + 1], cmr[:],
                                           Alu.mult, Alu.mult)  # m
            ovs = pp.tile([P, F], f32, tag="ovs")
            nc.vector.tensor_mul(ovs[:], B[:], nonself[:])
            nc.vector.tensor_reduce(
                ovred[:, t * NJ + jt:t * NJ + jt + 1], ovs[:], Axis.X, Alu.max)

            # --- align ---
            ax = pp.tile([P, F], f32, tag="ax")
            nc.scalar.activation(ax[:], ajr[:], Act.Abs,
                                 bias=neg_a[:, t:t + 1])
            pmx = pp.tile([P, F], f32, tag="pmx")
            nc.vector.tensor_scalar(pmx[:], ax[:], -1.0, PI, Alu.mult, Alu.add)
            nc.vector.tensor_tensor(ax[:], ax[:], pmx[:], Alu.min)   # amin
            nc.vector.tensor_scalar(ax[:], ax[:], 4.0 / PI, -1.0,
                                    Alu.mult, Alu.add)       # align0
            g32 = pp.tile([P, F], f32, tag="g32")
            nc.vector.tensor_scalar(g32[:], sq0[:], 1024.0, None, Alu.is_lt)
            nc.vector.tensor_mul(g32[:], g32[:], nonself[:])  # gate
            als = pp.tile([P, F], f32, tag="als")
            nc.vector.tensor_mul(als[:], ax[:], g32[:])
            nc.vector.tensor_reduce(
                alred[:, t * NJ + jt:t * NJ + jt + 1], als[:], Axis.X, Alu.min)

    # reduce over j-tiles
    ovmax = sg.tile([P, NT], f32)
    almin = sg.tile([P, NT], f32)
    ovv = _ap(ovred[:], ovred[:].offset, [ovred[:].ap[0], [NJ, NT], [1, NJ]])
    alv = _ap(alred[:], alred[:].offset, [alred[:].ap[0], [NJ, NT], [1, NJ]])
    nc.vector.tensor_reduce(ovmax[:], ovv, Axis.X, Alu.max)
    nc.vector.tensor_reduce(almin[:], alv, Axis.X, Alu.min)

    # ---------------- interp + combine ----------------
    cw_rep = sg.tile([P, 8], f32)
    nc.sync.dma_start(out=cw_rep[:], in_=_ap(cw_d, 0, [[0, P], [1, 8]]))
    e7 = sg.tile([P, NT, 8], f32)
    nc.vector.memset(e7[:, :, 7], 1.0)   # picks up comb_b from cw_rep[7]
    en = sg.tile([P, NT], f32)

    for t in range(NT):
        gt = gath[:, t, :]
        gv = _ap(gt, gt.offset, [gt.ap[0], [4, 7], [1, 4]])
        wv = _ap(Wt[:], Wt[:].offset + t * 4, [Wt[:].ap[0], [0, 7], [1, 4]])
        prod = sc.tile([P, 7, 4], f32, tag="prod")
        nc.vector.tensor_tensor(prod[:], gv, wv, Alu.mult)
        vals = sc.tile([P, 7], f32, tag="vals")
        nc.vector.tensor_reduce(vals[:], prod[:], Axis.X, Alu.add)
        # pos
        nc.vector.tensor_copy(e7[:, t:t + 1, 0:1], vals[:, 0:1])
        # width/length/angle: vA + cw*(vB-vA)
        for (ei, cwt, ia, ib) in ((1, cw_w, 1, 2), (2, cw_l, 3, 4),
                                  (3, cw_a, 5, 6)):
            dv = sc.tile([P, 1], f32, tag="dv")
            nc.vector.tensor_sub(dv[:], vals[:, ib:ib + 1], vals[:, ia:ia + 1])
            nc.vector.scalar_tensor_tensor(e7[:, t:t + 1, ei:ei + 1], dv[:],
                                           cwt[:, t:t + 1],
                                           vals[:, ia:ia + 1],
                                           Alu.mult, Alu.add)
        nc.vector.tensor_copy(e7[:, t:t + 1, 4:5], ovmax[:, t:t + 1])
        nc.vector.tensor_copy(e7[:, t:t + 1, 5:6], almin[:, t:t + 1])
        nc.vector.tensor_copy(e7[:, t:t + 1, 6:7], ae[:, t:t + 1])
        scr8 = sc.tile([P, 8], f32, tag="scr8")
        csum = sc.tile([P, 1], f32, tag="csum")
        nc.vector.tensor_mul(scr8[:], e7[:, t, :], cw_rep[:])
        nc.vector.tensor_reduce(csum[:], scr8[:], Axis.X, Alu.add)
        nc.vector.tensor_single_scalar(en[:, t:t + 1], csum[:],
                                       pm_sb[:, t:t + 1], Alu.mult)

    nc.sync.dma_start(out=en_d.rearrange("(p t) -> p t", t=NT), in_=en[:])
    rsum = sc.tile([P, 1], f32, tag="rsum")
    nc.vector.tensor_reduce(rsum[:], en[:], Axis.X, Alu.add)
    esum_sb = sc.tile([1, 1], f32, tag="esum")
    nc.gpsimd.tensor_reduce(esum_sb[:], rsum[:], Axis.C, Alu.add)
    nc.sync.dma_start(out=es_d, in_=esum_sb[:])


# ---------------------------------------------------------------------------
# host side
# ---------------------------------------------------------------------------
_NC_CACHE = {}
LAST_RESULTS = None


def _get_nc():
    if "nc" not in _NC_CACHE:
        _NC_CACHE["nc"] = build_kernel()
    return _NC_CACHE["nc"]


def make_in_maps(context_cube, context_cube_mask, position_energy_map,
                 mark_width_map, mark_length_map, mark_angle_map,
                 comb_w, comb_b):
    cc = np.ascontiguousarray(np.asarray(context_cube, np.float32))
    mask = np.asarray(context_cube_mask)
    gmap = np.concatenate([
        np.asarray(position_energy_map, np.float32).ravel(),
        np.asarray(mark_width_map, np.float32).ravel(),
        np.asarray(mark_length_map, np.float32).ravel(),
        np.asarray(mark_angle_map, np.float32).ravel(),
        np.zeros(PLANE, np.float32),
    ])
    assert gmap.size == NMAP
    cw8 = np.concatenate([np.asarray(comb_w, np.float32).ravel(),
                          np.asarray(comb_b, np.float32).ravel()[:1]
                          if np.asarray(comb_b).size else
                          np.zeros(1, np.float32)])
    if cw8.size == 7:
        cw8 = np.concatenate([cw8, np.zeros(1, np.float32)])
    cw8 = np.ascontiguousarray(cw8, np.float32)
    in_maps = []
    for c in range(NCORES):
        b, h = c // 2, c % 2
        pts = np.ascontiguousarray(cc[b, 0, 0, h * NPTS:(h + 1) * NPTS])
        pmv = mask[b, 0, 0, h * NPTS:(h + 1) * NPTS].astype(np.float32)
        pm_shuf = np.ascontiguousarray(pmv.reshape(NT, P).T)
        ctx = np.ascontiguousarray(cc[b].reshape(9 * 1024, 5))
        cm = np.ascontiguousarray(mask[b].reshape(9 * 1024).astype(np.float32))
        in_maps.append(dict(pts=pts, pm=pm_shuf.reshape(-1), ctx=ctx, cm=cm,
                            gmap=gmap, cw=cw8))
    return in_maps


def assemble(results):
    B, N = 4, 1024
    energies = np.zeros((B, N), np.float32)
    per_subset = np.zeros(B, np.float32)
    for c in range(NCORES):
        b, h = c // 2, c % 2
        en = results[c]["energies"].reshape(P, NT).T.reshape(-1)
        energies[b, h * NPTS:(h + 1) * NPTS] = en
        per_subset[b] += results[c]["esum"].ravel()[0]
    total = np.float32(per_subset.sum())
    return energies, per_subset, total


def kernel(**inputs):
    global LAST_RESULTS
    from concourse import bass_utils
    nc = _get_nc()
    in_maps = make_in_maps(
        inputs["context_cube"], inputs["context_cube_mask"],
        inputs["position_energy_map"], inputs["mark_width_map"],
        inputs["mark_length_map"], inputs["mark_angle_map"],
        inputs["comb_w"], inputs["comb_b"])
    res = bass_utils.run_bass_kernel_spmd(nc, in_maps,
                                          core_ids=list(range(NCORES)))
    LAST_RESULTS = res
    return assemble(res.results)
